# revision 11
# baseline (speedup 1.0000x reference)
"""MultiHeadAttention (N=4, L=2048, D=1024, H=16) on 8 Trainium2 NeuronCores.

Sharding: data-parallel over batch N (4) x tensor-parallel over head-groups (2).
Core c handles batch n = c // 2, heads [8*hg, 8*hg+8) with hg = c % 2.
Each core computes q/k/v projections for its 8 heads, causal flash-style
attention in transposed-score layout (scores.T[key j, query i]), and a
row-parallel Wo partial output. Host sums the two partials per batch.

All matmuls run in fp32r (TF32: fp32 data, HW rounds mantissa, fp32
accumulate) at 1 cycle/row -- 4x faster than plain fp32 matmul.
"""

import sys

if "/opt/trn_rl_repo" not in sys.path:
    sys.path.insert(0, "/opt/trn_rl_repo")

import numpy as np

N, L, D, H = 4, 2048, 1024, 16
DK = D // H          # 64
N_CORES = 8
HLOC = H // 2        # heads per core: 8
MLOC = HLOC * DK     # local feature width: 512
IC = 256             # i-chunk (query) width for attention
NIC = L // IC        # 8 i-chunks
NJT = L // 128       # 16 key tiles
LC = 512             # l-chunk width for projections
NLC = L // LC        # 4
NMT = MLOC // 128    # 4 m-tiles
NDT = D // 128       # 8 d-tiles

_cache = {}


def _build(causal: bool, debug: bool = False):
    import concourse.tile as tile
    from concourse import bacc, mybir

    F32, F32R = mybir.dt.float32, mybir.dt.float32r
    EXPF = mybir.ActivationFunctionType.Exp
    scale = 1.0 / float(np.sqrt(DK))

    nc = bacc.Bacc("TRN2", target_bir_lowering=False, debug=False,
                   num_devices=N_CORES)
    qt_d = nc.dram_tensor("qt", [D, L], F32R, kind="ExternalInput").ap()
    kt_d = nc.dram_tensor("kt", [D, L], F32R, kind="ExternalInput").ap()
    vt_d = nc.dram_tensor("vt", [D, L], F32R, kind="ExternalInput").ap()
    wqt_d = nc.dram_tensor("wqt", [D, MLOC], F32R, kind="ExternalInput").ap()
    wkt_d = nc.dram_tensor("wkt", [D, MLOC], F32R, kind="ExternalInput").ap()
    wvt_d = nc.dram_tensor("wvt", [D, MLOC], F32R, kind="ExternalInput").ap()
    wot_d = nc.dram_tensor("wot", [MLOC, D], F32R, kind="ExternalInput").ap()
    if not causal:
        msk_d = nc.dram_tensor("msk", [L, L], F32R, kind="ExternalInput").ap()
    out_d = nc.dram_tensor("out", [L, D], F32, kind="ExternalOutput").ap()
    if debug:
        dq_d = nc.dram_tensor("dq", [NMT, 128, L], F32R, kind="ExternalOutput").ap()
        dk_d = nc.dram_tensor("dk", [NMT, 128, L], F32R, kind="ExternalOutput").ap()
        dv_d = nc.dram_tensor("dv", [L // 128, 128, HLOC, DK + 1], F32R, kind="ExternalOutput").ap()
        dctx_d = nc.dram_tensor("dctx", [HLOC, 64, L], F32R, kind="ExternalOutput").ap()
        dsum_d = nc.dram_tensor("dsum", [1, NIC * IC], F32, kind="ExternalOutput").ap()
        dpt_d = nc.dram_tensor("dpt", [128, 2, IC], F32R, kind="ExternalOutput").ap()

    with tile.TileContext(nc) as tc:
        # qT/kT/vE live through phases A+B; ctx lives through B+C.
        pab_cm = tc.tile_pool(name="pab", bufs=1)
        pab = pab_cm.__enter__()
        qT = [pab.tile([128, L], F32R, tag=f"qT{m}", name=f"qT{m}")
              for m in range(NMT)]
        kT = [pab.tile([128, L], F32R, tag=f"kT{m}", name=f"kT{m}")
              for m in range(NMT)]
        # v per l-tile: [l=128, head, dk+1]; last col = ones (softmax denom)
        vE = [pab.tile([128, HLOC, DK + 1], F32R, tag=f"v{lt}", name=f"v{lt}")
              for lt in range(L // 128)]
        ones8 = pab.tile([128, HLOC, 1], F32, tag="ones8", name="ones8")
        nc.vector.memset(ones8[:], 1.0)

        # ---- phase A: projections ----
        with tc.tile_pool(name="wpool", bufs=1) as wp, \
             tc.tile_pool(name="astream", bufs=2) as astream, \
             tc.tile_pool(name="aps", bufs=3, space="PSUM") as aps:
            wq = wp.tile([128, NDT, MLOC], F32R, tag="wq")
            wk = wp.tile([128, NDT, MLOC], F32R, tag="wk")
            wv = wp.tile([128, NDT, MLOC], F32R, tag="wv")
            nc.sync.dma_start(wq[:], wqt_d.rearrange("(dt p) m -> p dt m", p=128))
            nc.sync.dma_start(wk[:], wkt_d.rearrange("(dt p) m -> p dt m", p=128))
            nc.sync.dma_start(wv[:], wvt_d.rearrange("(dt p) m -> p dt m", p=128))
            qt_r = qt_d.rearrange("(dt p) l -> p dt l", p=128)
            kt_r = kt_d.rearrange("(dt p) l -> p dt l", p=128)
            vt_r = vt_d.rearrange("(dt p) l -> p dt l", p=128)

            # q, k: feature-major out [m-tile 128, l 512]
            for w_sb, x_r, dst, xtag in ((wq, qt_r, qT, "xq"),
                                         (wk, kt_r, kT, "xk")):
                for lc in range(NLC):
                    ls = slice(lc * LC, (lc + 1) * LC)
                    x_sb = astream.tile([128, NDT, LC], F32R, tag="x",
                                        name=xtag)
                    nc.sync.dma_start(x_sb[:], x_r[:, :, ls])
                    for mt in range(NMT):
                        ps = aps.tile([128, LC], F32, tag="proj", name="psp")
                        for dt in range(NDT):
                            nc.tensor.matmul(
                                ps[:], w_sb[:, dt, mt * 128:(mt + 1) * 128],
                                x_sb[:, dt, :],
                                start=(dt == 0), stop=(dt == NDT - 1))
                        nc.vector.tensor_copy(dst[mt][:, ls], ps[:])
            # v: token-major out [l-tile 128, m 512]
            for lc in range(NLC):
                ls = slice(lc * LC, (lc + 1) * LC)
                xv = astream.tile([128, NDT, LC], F32R, tag="x", name="xv")
                nc.sync.dma_start(xv[:], vt_r[:, :, ls])
                for li in range(LC // 128):
                    lt = lc * (LC // 128) + li
                    ps = aps.tile([128, MLOC], F32, tag="proj", name="psv")
                    for dt in range(NDT):
                        nc.tensor.matmul(
                            ps[:], xv[:, dt, li * 128:(li + 1) * 128],
                            wv[:, dt, :],
                            start=(dt == 0), stop=(dt == NDT - 1))
                    nc.vector.tensor_copy(
                        vE[lt][:, :, 0:DK],
                        ps[:].rearrange("p (h d) -> p h d", h=HLOC))
                    nc.vector.tensor_copy(vE[lt][:, :, DK:DK + 1], ones8[:])

        if debug:
            for mt in range(NMT):
                nc.sync.dma_start(dq_d[mt], qT[mt][:])
                nc.sync.dma_start(dk_d[mt], kT[mt][:])
            for lt in range(L // 128):
                nc.sync.dma_start(dv_d[lt], vE[lt][:])

        pbc_cm = tc.tile_pool(name="pbc", bufs=1, side="right")
        pbc = pbc_cm.__enter__()
        ctx = [pbc.tile([64, L], F32R, tag=f"ctx{h}", name=f"ctx{h}")
               for h in range(HLOC)]

        # ---- phase B: attention ----
        with tc.tile_pool(name="battn", bufs=4) as bp, \
             tc.tile_pool(name="bmask", bufs=1) as bmp, \
             tc.tile_pool(name="bmload", bufs=2) as bml, \
             tc.tile_pool(name="bnorm", bufs=4) as bnp, \
             tc.tile_pool(name="sps", bufs=3, space="PSUM") as sps, \
             tc.tile_pool(name="cps", bufs=4, space="PSUM") as cps:
            if causal:
                ones_t = bmp.tile([128, IC], F32, tag="ones", name="ones")
                nc.vector.memset(ones_t[:], 1.0)
                m01 = []
                for di, base in enumerate((0, -128)):
                    mt_ = bmp.tile([128, IC], F32, tag=f"m01_{di}",
                                   name=f"m01_{di}")
                    nc.gpsimd.affine_select(
                        mt_[:], ones_t[:], pattern=[[1, IC]],
                        compare_op=mybir.AluOpType.is_ge, fill=0.0,
                        base=base, channel_multiplier=-1)
                    m01.append(mt_)

            for h in range(HLOC):
                mt4, po = h // 2, 64 * (h % 2)
                hs = slice(po, po + 64)
                # i-chunks in two half-passes of 4 so each ctx accumulator
                # owns a full PSUM bank (start=True clears the whole bank)
                for half in range(2):
                    ics = list(range(half * (NIC // 2), (half + 1) * (NIC // 2)))
                    cpss = {ic: cps.tile([DK + 1, IC], F32, tag="ctxp",
                                         name="ctxp") for ic in ics}
                    jt_max = 2 * ics[-1] + 1 if causal else NJT - 1
                    for jt in range(jt_max + 1):
                        js = slice(jt * 128, (jt + 1) * 128)
                        if not causal:
                            mrow = bml.tile([128, L], F32R, tag="mrow",
                                            name="mrow")
                            nc.sync.dma_start(mrow[:], msk_d[js, :])
                        for ic in ics:
                            if causal and ic < jt // 2:
                                continue
                            isl = slice(ic * IC, (ic + 1) * IC)
                            sp = sps.tile([128, IC], F32, tag="scores",
                                          name="scores")
                            nc.tensor.matmul(
                                sp[:], kT[mt4][hs, js], qT[mt4][hs, isl],
                                start=True, stop=True, tile_position=(po, 0))
                            pt = bp.tile([128, IC], F32R, tag="pt", name="pt")
                            nc.scalar.activation(pt[:], sp[:], EXPF,
                                                 scale=scale)
                            if causal:
                                if jt == 2 * ic or jt == 2 * ic + 1:
                                    nc.vector.tensor_mul(
                                        pt[:], pt[:], m01[jt - 2 * ic][:])
                            else:
                                nc.vector.tensor_mul(pt[:], pt[:],
                                                     mrow[:, isl])
                            if debug and h == 0 and ic == 0 and jt < 2:
                                nc.sync.dma_start(dpt_d[:, jt, :], pt[:])
                            nc.tensor.matmul(
                                cpss[ic][:], vE[jt][:, h, :], pt[:],
                                start=(jt == 0),
                                stop=(jt == (2 * ic + 1 if causal
                                             else NJT - 1)))
                    for ic in ics:
                        isl = slice(ic * IC, (ic + 1) * IC)
                        rr = bnp.tile([1, IC], F32, tag="rrow", name="rrow")
                        if debug and h == 0:
                            dsum_sb = bnp.tile([1, IC], F32, tag="dsum",
                                               name="dsum_sb")
                            nc.vector.tensor_copy(dsum_sb[:],
                                                  cpss[ic][DK:DK + 1, :])
                            nc.sync.dma_start(dsum_d[:, isl], dsum_sb[:])
                        nc.vector.reciprocal(rr[:], cpss[ic][DK:DK + 1, :])
                        rb = bnp.tile([64, IC], F32, tag="rb", name="rb")
                        nc.gpsimd.partition_broadcast(rb[:], rr[:])
                        nc.vector.tensor_mul(ctx[h][:, isl],
                                             cpss[ic][0:DK, :], rb[:])

        if debug:
            for h in range(HLOC):
                nc.sync.dma_start(dctx_d[h], ctx[h][:])

        pab_cm.__exit__(None, None, None)

        # ---- phase C: output projection (row-parallel partial) ----
        with tc.tile_pool(name="cw", bufs=1) as cw, \
             tc.tile_pool(name="cout", bufs=3) as co, \
             tc.tile_pool(name="ops", bufs=2, space="PSUM") as ops:
            wo = cw.tile([64, HLOC, D], F32R, tag="wo")
            nc.sync.dma_start(wo[:], wot_d.rearrange("(h p) e -> p h e", p=64))
            for lt in range(L // 128):
                lsl = slice(lt * 128, (lt + 1) * 128)
                ot = co.tile([128, D], F32, tag="ot", name="ot")
                for et in range(2):
                    esl = slice(et * 512, (et + 1) * 512)
                    ps = ops.tile([128, 512], F32, tag="wops", name="wops")
                    for h in range(HLOC):
                        nc.tensor.matmul(
                            ps[:], ctx[h][:, lsl], wo[:, h, esl],
                            start=(h == 0), stop=(h == HLOC - 1))
                    nc.vector.tensor_copy(ot[:, esl], ps[:])
                nc.sync.dma_start(out_d[lsl, :], ot[:])

        pbc_cm.__exit__(None, None, None)

    nc.compile()
    return nc


def kernel(Q, K, V, Wq, Wk, Wv, Wo, attn_mask, key_padding_mask):
    from concourse.bass_utils import run_bass_kernel_spmd

    Q = np.asarray(Q, dtype=np.float32)
    K = np.asarray(K, dtype=np.float32)
    V = np.asarray(V, dtype=np.float32)
    Wq = np.asarray(Wq, dtype=np.float32)
    Wk = np.asarray(Wk, dtype=np.float32)
    Wv = np.asarray(Wv, dtype=np.float32)
    Wo = np.asarray(Wo, dtype=np.float32)
    am = np.asarray(attn_mask).astype(bool)
    kpm = np.asarray(key_padding_mask).astype(bool)

    causal = bool(np.array_equal(am, np.tril(np.ones((L, L), dtype=bool)))
                  and kpm.all())
    if _cache.get("causal") != causal:
        _cache["nc"] = _build(causal)
        _cache["causal"] = causal
    nc = _cache["nc"]

    in_maps = []
    for c in range(N_CORES):
        n, hg = c // 2, c % 2
        ms = slice(hg * MLOC, (hg + 1) * MLOC)
        m = {
            "qt": np.ascontiguousarray(Q[n].T),
            "kt": np.ascontiguousarray(K[n].T),
            "vt": np.ascontiguousarray(V[n].T),
            "wqt": np.ascontiguousarray(Wq[ms, :].T),
            "wkt": np.ascontiguousarray(Wk[ms, :].T),
            "wvt": np.ascontiguousarray(Wv[ms, :].T),
            "wot": np.ascontiguousarray(Wo[:, ms].T),
        }
        if not causal:
            # multiplicative 0/1 mask, transposed layout [key j, query i]:
            # valid(i, j) = attn_mask[i, j] & key_padding[n, j]
            m["msk"] = np.ascontiguousarray(
                (am & kpm[n][None, :]).T.astype(np.float32))
        in_maps.append(m)

    res = run_bass_kernel_spmd(nc, in_maps, core_ids=list(range(N_CORES)))
    _cache["last_in_maps"] = in_maps

    out = np.empty((N, L, D), dtype=np.float32)
    for n in range(N):
        out[n] = res.results[2 * n]["out"] + res.results[2 * n + 1]["out"]
    return out
